# revision 1
# baseline (speedup 1.0000x reference)
"""Multi-head self-attention (B=4, T=2048, C=1024, H=16) on 8 Trainium2 cores.

Sharding: core c handles (batch b = c//2, query-half qh = c%2). Each core
computes K/V for all 2048 keys of its batch (redundant within the pair) and
attention + output projection for its 1024 query rows — no collectives.

Per-core input x is pre-rotated on host so the core's query half is always
rows 0:1024 (key order is permuted identically for both halves; softmax
attention is permutation-invariant over keys, so outputs are unaffected).

Matmul operands are bf16 (PE streams 1 column/cycle; fp32 moving operands
measured at half rate), accumulation is fp32 in PSUM. Everything stays in
SBUF in transposed "feature-on-partition" layout:
  X^T (c, t) via hardware DMA-transpose;
  V (t, [v_h | 1] x 16 heads) so each AV stationary tile carries a ones
  column and softmax denominators ride along as output row 64;
  K^T/Q^T (f, t) computed per head-pair, interleaved with attention;
  S^T (k, q) per head;  P^T = exp(S^T/8) on ScalarE;
  normalize with reciprocal_approx + K=1 broadcast matmul;
  project with q-rows as stationary M so output lands q-major.
"""
import sys

sys.path.insert(0, "/opt/trn_rl_repo")

from contextlib import ExitStack

import numpy as np

import concourse.bacc as bacc
import concourse.tile as tile
from concourse import mybir
from concourse.bass_utils import run_bass_kernel_spmd

F32 = mybir.dt.float32
BF16 = mybir.dt.bfloat16
AF = mybir.ActivationFunctionType

T, C, NH, D = 2048, 1024, 16, 64
TQ = T // 2            # queries per core
P = 128
N_KC = C // P          # 8 contraction chunks
N_M = C // P           # 8 feature chunks
N_TT = T // P          # 16 token chunks (keys)
N_HP = NH // 2         # 8 head pairs
N_QM = TQ // P         # 8 query chunks
VW = 65                # per-head V width incl. ones column

_CACHE = {}


def _build(debug=False):
    nc = bacc.Bacc("TRN2", target_bir_lowering=False, debug=False)

    x = nc.declare_dram_parameter("x", [T, C], BF16, isOutput=False)
    wq = nc.declare_dram_parameter("wq", [C, C], BF16, isOutput=False)
    wk = nc.declare_dram_parameter("wk", [C, C], BF16, isOutput=False)
    wv = nc.declare_dram_parameter("wv", [C, C], BF16, isOutput=False)
    wo = nc.declare_dram_parameter("wo", [C, C], BF16, isOutput=False)
    bq = nc.declare_dram_parameter("bq", [C], F32, isOutput=False)
    bv_b = nc.declare_dram_parameter("bv_b", [P, C], F32, isOutput=False)
    bo_b = nc.declare_dram_parameter("bo_b", [P, C], F32, isOutput=False)
    out = nc.declare_dram_parameter("out", [N_QM, P, C], F32, isOutput=True)

    dbg = {}
    if debug:
        for name, shape, dt_ in [
            ("dbg_xt", [P, TQ], BF16),      # X^T query-half chunk kc=0
            ("dbg_qt", [P, TQ], BF16),      # Q^T chunk m=0
            ("dbg_kt", [P, T], BF16),       # K^T chunk m=0
            ("dbg_vd", [P, C], BF16),       # V chunk tt=0 (heads de-interleaved)
            ("dbg_pt", [P, TQ], BF16),      # P^T hp=0 kt=0 head A
            ("dbg_o", [VW, TQ], F32),       # O^T hp=0 head A
            ("dbg_rc", [1, TQ], F32),       # recip row hp=0 head A
            ("dbg_bc", [64, TQ], F32),      # bcast tile hp=0 head A
            ("dbg_ao", [P, TQ], BF16),      # attout chunk kc=0
        ]:
            dbg[name] = nc.declare_dram_parameter(name, shape, dt_, isOutput=True)

    with tile.TileContext(nc) as tc, ExitStack() as ctx:
        const = ctx.enter_context(tc.tile_pool(name="const", bufs=1))
        big = ctx.enter_context(tc.tile_pool(name="big", bufs=1))
        ps_mm = ctx.enter_context(tc.tile_pool(name="psmm", bufs=2, space="PSUM"))

        ones32 = const.tile([1, P], F32)
        nc.vector.memset(ones32[:, :], 1.0)
        bq_t = const.tile([P, N_M], F32)
        nc.sync.dma_start(out=bq_t[:, :], in_=bq.rearrange("(m p) -> p m", p=P))
        bv_t = const.tile([P, C], F32)
        nc.sync.dma_start(out=bv_t[:, :], in_=bv_b[:, :])
        bo_t = const.tile([P, C], F32)
        nc.sync.dma_start(out=bo_t[:, :], in_=bo_b[:, :])

        xt_q = big.tile([P, N_KC, TQ], BF16)        # X^T query half (c, q)
        qt = big.tile([P, N_M, TQ], BF16)           # Q^T (f, q)
        kt_res = big.tile([P, N_M, T], BF16)        # K^T (f, t)
        v_res = big.tile([P, N_TT, NH * VW], BF16)  # [v_h | 1] per head, per t-chunk
        attout = big.tile([P, N_M, TQ], BF16)       # att output^T

        # ones columns of v_res: one strided memset
        v_ones = v_res.rearrange("p t (h w) -> p t h w", w=VW)
        nc.vector.memset(v_ones[:, :, :, 64:65], 1.0)

        with ExitStack() as p2:
            xt_pool = p2.enter_context(tc.tile_pool(name="xt", bufs=1))
            w_pool = p2.enter_context(tc.tile_pool(name="w", bufs=9))

            # ---- Phase 1: X^T via hardware DMA transpose (sync queue) ---------
            xt = xt_pool.tile([P, N_KC, T], BF16)   # X^T (c, t)
            for kc in range(N_KC):
                nc.sync.dma_start(
                    out=xt[:, kc, :], in_=x[:, kc * P : (kc + 1) * P], transpose=True
                )
                nc.sync.dma_start(
                    out=xt_q[:, kc, :],
                    in_=x[0:TQ, kc * P : (kc + 1) * P],
                    transpose=True,
                )

            # ---- Phase 2a: V = X @ Wv + bv, heads interleaved with ones -------
            wt = []
            for kc in range(N_KC):
                w_t = w_pool.tile([P, C], BF16, tag="w")
                nc.scalar.dma_start(out=w_t[:, :], in_=wv[kc * P : (kc + 1) * P, :])
                wt.append(w_t)
            for tt in range(N_TT):
                pv = ps_mm.tile([P, C], F32, tag="mm")
                for kc in range(N_KC):
                    for nh in range(2):
                        nc.tensor.matmul(
                            pv[:, nh * 512 : (nh + 1) * 512],
                            xt[:, kc, tt * P : (tt + 1) * P],
                            wt[kc][:, nh * 512 : (nh + 1) * 512],
                            start=(kc == 0),
                            stop=(kc == N_KC - 1),
                        )
                pv_v = pv.rearrange("p (h f) -> p h f", h=NH)
                bv_v = bv_t.rearrange("p (h f) -> p h f", h=NH)
                nc.vector.tensor_add(
                    v_ones[:, tt, :, 0:64], pv_v[:, :, :], bv_v[:, :, :]
                )

            # ---- Phase 2b: K^T = (X @ Wk)^T (bk cancels in softmax) -----------
            wt = []
            for kc in range(N_KC):
                w_t = w_pool.tile([P, C], BF16, tag="w")
                nc.scalar.dma_start(out=w_t[:, :], in_=wk[kc * P : (kc + 1) * P, :])
                wt.append(w_t)
            for m in range(N_M):
                for th in range(2):
                    pk = ps_mm.tile([P, C], F32, tag="mm")
                    for kc in range(N_KC):
                        for nh in range(2):
                            nc.tensor.matmul(
                                pk[:, nh * 512 : (nh + 1) * 512],
                                wt[kc][:, m * P : (m + 1) * P],
                                xt[:, kc, th * 1024 + nh * 512 : th * 1024 + (nh + 1) * 512],
                                start=(kc == 0),
                                stop=(kc == N_KC - 1),
                            )
                    nc.vector.tensor_copy(
                        kt_res[:, m, th * 1024 : (th + 1) * 1024], pk[:, :]
                    )

        # ---- Phase 3: per head pair: Q^T (full-array warm work) + attention ---
        with ExitStack() as p3:
            wq_pool = p3.enter_context(tc.tile_pool(name="wqs", bufs=2))
            pt_pool = p3.enter_context(tc.tile_pool(name="pt", bufs=4))
            rc_pool = p3.enter_context(tc.tile_pool(name="rc", bufs=2))
            bc_pool = p3.enter_context(tc.tile_pool(name="bc", bufs=2))
            ps_o = p3.enter_context(tc.tile_pool(name="pso", bufs=2, space="PSUM"))

            wq_r = wq.rearrange("(kc p) f -> kc p f", p=P)
            for hp in range(N_HP):
                # Q^T rows for this head pair
                wq_hp = wq_pool.tile([P, N_KC, P], BF16, tag="wqs")
                nc.scalar.dma_start(
                    out=wq_hp[:, :, :], in_=wq_r[:, :, hp * P : (hp + 1) * P].rearrange("kc p f -> p kc f")
                )
                pq = ps_mm.tile([P, C], F32, tag="mm")
                for kc in range(N_KC):
                    for nh in range(2):
                        nc.tensor.matmul(
                            pq[:, nh * 512 : (nh + 1) * 512],
                            wq_hp[:, kc, :],
                            xt_q[:, kc, nh * 512 : (nh + 1) * 512],
                            start=(kc == 0),
                            stop=(kc == N_KC - 1),
                        )
                nc.vector.tensor_scalar_add(
                    qt[:, hp, :], pq[:, :], bq_t[:, hp : hp + 1]
                )
                if debug and hp == 0:
                    nc.sync.dma_start(out=dbg["dbg_qt"][:, :], in_=qt[:, 0, :])
                    nc.sync.dma_start(out=dbg["dbg_xt"][:, :], in_=xt_q[:, 0, :])
                    nc.sync.dma_start(out=dbg["dbg_kt"][:, :], in_=kt_res[:, 0, :])
                    dv = v_res.rearrange("p t (h w) -> p t h w", w=VW)[
                        :, 0, :, 0:64
                    ]
                    nc.sync.dma_start(out=dbg["dbg_vd"][:, :], in_=dv)

                oA = ps_o.tile([VW, TQ], F32, tag="o")
                oB = ps_o.tile([VW, TQ], F32, tag="o")
                for kt in range(N_TT):
                    sA = ps_mm.tile([P, TQ], F32, tag="mm")
                    sB = ps_mm.tile([P, TQ], F32, tag="mm")
                    for qc in range(2):
                        nc.tensor.matmul(
                            sA[:, qc * 512 : (qc + 1) * 512],
                            kt_res[0:64, hp, kt * P : (kt + 1) * P],
                            qt[0:64, hp, qc * 512 : (qc + 1) * 512],
                            start=True,
                            stop=True,
                            tile_position=(0, 0),
                        )
                        nc.tensor.matmul(
                            sB[:, qc * 512 : (qc + 1) * 512],
                            kt_res[64:128, hp, kt * P : (kt + 1) * P],
                            qt[64:128, hp, qc * 512 : (qc + 1) * 512],
                            start=True,
                            stop=True,
                            tile_position=(64, 0),
                        )
                    ptA = pt_pool.tile([P, TQ], BF16, tag="pt")
                    nc.scalar.activation(ptA[:, :], sA[:, :], AF.Exp, scale=0.125)
                    ptB = pt_pool.tile([P, TQ], BF16, tag="pt")
                    nc.scalar.activation(ptB[:, :], sB[:, :], AF.Exp, scale=0.125)
                    if debug and hp == 0 and kt == 0:
                        nc.sync.dma_start(out=dbg["dbg_pt"][:, :], in_=ptA[:, :])
                    for qc in range(2):
                        nc.tensor.matmul(
                            oA[:, qc * 512 : (qc + 1) * 512],
                            v_res[:, kt, 2 * hp * VW : (2 * hp + 1) * VW],
                            ptA[:, qc * 512 : (qc + 1) * 512],
                            start=(kt == 0),
                            stop=(kt == N_TT - 1),
                        )
                        nc.tensor.matmul(
                            oB[:, qc * 512 : (qc + 1) * 512],
                            v_res[:, kt, (2 * hp + 1) * VW : (2 * hp + 2) * VW],
                            ptB[:, qc * 512 : (qc + 1) * 512],
                            start=(kt == 0),
                            stop=(kt == N_TT - 1),
                        )

                if debug and hp == 0:
                    o_dbg = bc_pool.tile([VW, TQ], F32, tag="odbg")
                    nc.vector.tensor_copy(o_dbg[:, :], oA[:, :])
                    nc.sync.dma_start(out=dbg["dbg_o"][:, :], in_=o_dbg[:, :])

                # normalize: att^T[d, q] = O^T[d, q] / O^T[64, q]
                for row0, o_ps in ((0, oA), (64, oB)):
                    den = rc_pool.tile([1, TQ], F32, tag="den")
                    nc.vector.tensor_copy(den[:, :], o_ps[64:65, :])
                    rc = rc_pool.tile([1, TQ], F32, tag="rc")
                    nc.vector.reciprocal_approx_fast(out=rc[:, :], in_=den[:, :])
                    bc_ps = ps_mm.tile([P, TQ], F32, tag="mm")
                    for qc in range(2):
                        nc.tensor.matmul(
                            bc_ps[0:64, qc * 512 : (qc + 1) * 512],
                            ones32[:, 0:64],
                            rc[:, qc * 512 : (qc + 1) * 512],
                            start=True,
                            stop=True,
                        )
                    bc_sb = bc_pool.tile([64, TQ], F32, tag="bc")
                    nc.vector.tensor_copy(bc_sb[:, :], bc_ps[0:64, :])
                    nc.vector.tensor_mul(
                        attout[row0 : row0 + 64, hp, :],
                        o_ps[0:64, :],
                        bc_sb[:, :],
                    )
                    if debug and hp == 0 and row0 == 0:
                        nc.sync.dma_start(out=dbg["dbg_rc"][:, :], in_=rc[:, :])
                        nc.sync.dma_start(out=dbg["dbg_bc"][:, :], in_=bc_sb[:, :])

        if debug:
            nc.sync.dma_start(out=dbg["dbg_ao"][:, :], in_=attout[:, 0, :])

        # ---- Phase 4: output projection (q-major output) ----------------------
        with ExitStack() as p4:
            w_pool = p4.enter_context(tc.tile_pool(name="w2", bufs=9))
            st_pool = p4.enter_context(tc.tile_pool(name="st2", bufs=3))
            wt = []
            for kc in range(N_KC):
                w_t = w_pool.tile([P, C], BF16, tag="w2")
                nc.scalar.dma_start(out=w_t[:, :], in_=wo[kc * P : (kc + 1) * P, :])
                wt.append(w_t)
            for qm in range(N_QM):
                po = ps_mm.tile([P, C], F32, tag="mm")
                for kc in range(N_KC):
                    for nh in range(2):
                        nc.tensor.matmul(
                            po[:, nh * 512 : (nh + 1) * 512],
                            attout[:, kc, qm * P : (qm + 1) * P],
                            wt[kc][:, nh * 512 : (nh + 1) * 512],
                            start=(kc == 0),
                            stop=(kc == N_KC - 1),
                        )
                os_ = st_pool.tile([P, C], F32, tag="st2")
                nc.vector.tensor_add(os_[:, :], po[:, :], bo_t[:, :])
                nc.sync.dma_start(out=out[qm, :, :], in_=os_[:, :])

    nc.finalize()
    return nc


def _get_program():
    if "nc" not in _CACHE:
        _CACHE["nc"] = _build()
    return _CACHE["nc"]


def _bf16(a):
    import ml_dtypes

    return np.asarray(a, np.float32).astype(ml_dtypes.bfloat16)


def kernel(x, Wq, bq, Wk, bk, Wv, bv, Wo, bo, _trace=False, _trace_kwargs=None):
    x = np.asarray(x, np.float32)
    bq, bv, bo = (np.asarray(b, np.float32) for b in (bq, bv, bo))
    wq_b, wk_b, wv_b, wo_b = _bf16(Wq), _bf16(Wk), _bf16(Wv), _bf16(Wo)
    # bk unused: a key-side bias adds a per-query constant to every logit of a
    # softmax row, which cancels exactly in the softmax.

    bv_bc = np.ascontiguousarray(np.broadcast_to(bv, (P, C)))
    bo_bc = np.ascontiguousarray(np.broadcast_to(bo, (P, C)))

    nc = _get_program()
    in_maps = []
    for c in range(8):
        b, qh = divmod(c, 2)
        if qh == 0:
            x_in = x[b]
        else:
            x_in = np.concatenate([x[b, TQ:], x[b, :TQ]], axis=0)
        in_maps.append(
            {
                "x": _bf16(x_in),
                "wq": wq_b, "wk": wk_b, "wv": wv_b, "wo": wo_b,
                "bq": bq, "bv_b": bv_bc, "bo_b": bo_bc,
            }
        )

    kw = {}
    if _trace:
        kw = dict(trace=True, **(_trace_kwargs or {}))
    res = run_bass_kernel_spmd(nc, in_maps, list(range(8)), **kw)
    _CACHE["last_result"] = res

    out = np.empty((4, T, C), np.float32)
    for c in range(8):
        b, qh = divmod(c, 2)
        out[b, qh * TQ : (qh + 1) * TQ] = res.results[c]["out"].reshape(TQ, C)
    return out



# revision 18
# speedup vs baseline: 1.2827x; 1.2827x over previous
"""Multi-head self-attention (B=4, T=2048, C=1024, H=16) on 8 Trainium2 cores.

Sharding v2 (head-split): core c handles batch b = c//2 and head-half
hh = c%2 (8 of the 16 heads), ALL 2048 queries and keys of its batch.
No K/V projection redundancy. The output projection contracts only this
core's 512 feature columns, so each core returns a PARTIAL [2048, 1024]
fp32 product; the host sums the two partials per batch and adds bo.

Per-core engine plan (measured constants: bf16 N=512 matmul streams
back-to-back at 216 ns with LDWEIGHTS fully hidden; K=64 matmul pairs at
tile_position (0,0)/(64,0) run CONCURRENTLY, i.e. an S-pair costs one
matmul; ScalarE ACTIVATE = (N+352)/1.2 ns):
  - PE: V/K/Q projections (384 MMs), S pairs + AV (1024 MMs), out-proj
    (128 MMs).
  - ScalarE: the 33.5M-element exp() in [128,1024] tiles - the pacer
    (~294 us floor).
  - DVE: bias adds, PSUM->SBUF casts, softmax normalize muls.
  - DMA: row-broadcast of the softmax reciprocal across 64 partitions.

Everything stays feature-on-partition: X^T via DMA transpose; K^T/Q^T
per head pair (2x64 features on partitions 0:63/64:127); V as
[key-chunk, head, 64+ones] so softmax denominators ride along row 64 of
the AV accumulation.
"""
import sys

sys.path.insert(0, "/opt/trn_rl_repo")

from contextlib import ExitStack

import numpy as np

import concourse.bacc as bacc
import concourse.tile as tile
from concourse import mybir
from concourse.bass_utils import run_bass_kernel_spmd

F32 = mybir.dt.float32
BF16 = mybir.dt.bfloat16
AF = mybir.ActivationFunctionType

T, C, NH, D = 2048, 1024, 16, 64
HH = 8                  # heads per core
HF = HH * D             # 512 feature columns per core
P = 128
N_KC = C // P           # 8 contraction chunks
N_TT = T // P           # 16 token/key chunks
N_HP = HH // 2          # 4 head pairs per core
N_QP = 4                # query passes of 512
QW = T // N_QP          # 512 queries per pass
VW = D + 1              # per-head V width incl. ones column

_CACHE = {}


def _build(debug=False):
    nc = bacc.Bacc("TRN2", target_bir_lowering=False, debug=False)

    x = nc.declare_dram_parameter("x", [T, C], BF16, isOutput=False)
    wq = nc.declare_dram_parameter("wq", [C, HF], BF16, isOutput=False)
    wk = nc.declare_dram_parameter("wk", [C, HF], BF16, isOutput=False)
    wv = nc.declare_dram_parameter("wv", [C, HF], BF16, isOutput=False)
    wo = nc.declare_dram_parameter("wo", [HF, C], BF16, isOutput=False)
    bq = nc.declare_dram_parameter("bq", [HF], F32, isOutput=False)
    bv_b = nc.declare_dram_parameter("bv_b", [P, HF], F32, isOutput=False)
    out = nc.declare_dram_parameter("out", [N_TT, P, C], F32, isOutput=True)

    dbg = {}
    if debug:
        for name, shape, dt_ in [
            ("dbg_xt", [P, T], BF16),       # X^T chunk kc=0
            ("dbg_qt", [P, T], BF16),       # Q^T hp=0
            ("dbg_kt", [P, T], BF16),       # K^T hp=0
            ("dbg_vd", [P, HH * VW], BF16), # v_res chunk tt=0
            ("dbg_s", [P, 2 * QW], F32),    # S hp=0 qp=0 kt=0
            ("dbg_pt", [P, 2 * QW], BF16),  # P hp=0 qp=0 kt=0
            ("dbg_o", [VW, QW], F32),       # O_A hp=0 qp=0
            ("dbg_rc", [1, QW], F32),       # recip hp=0 qp=0 head A
            ("dbg_bc", [64, QW], F32),      # bcast tile
            ("dbg_ao", [P, T], BF16),       # attout hp=0
        ]:
            dbg[name] = nc.declare_dram_parameter(name, shape, dt_, isOutput=True)

    with tile.TileContext(nc) as tc, ExitStack() as ctx:
        big = ctx.enter_context(tc.tile_pool(name="big", bufs=1))
        pt_pool = ctx.enter_context(tc.tile_pool(name="pt", bufs=3))
        rc_pool = ctx.enter_context(tc.tile_pool(name="rc", bufs=2))
        bc_pool = ctx.enter_context(tc.tile_pool(name="bc", bufs=2))
        s_ps = ctx.enter_context(tc.tile_pool(name="sps", bufs=2, space="PSUM"))
        o_ps = ctx.enter_context(tc.tile_pool(name="ops", bufs=2, space="PSUM"))
        pr_ps = ctx.enter_context(tc.tile_pool(name="prps", bufs=2, space="PSUM"))

        # ---- inputs to SBUF -------------------------------------------------
        xt = big.tile([P, N_KC, T], BF16)          # X^T (c, t)
        qdma = [nc.sync, nc.sync]
        for kc in range(N_KC):
            qdma[kc % 2].dma_start(
                out=xt[:, kc, :], in_=x[:, kc * P : (kc + 1) * P], transpose=True
            )

        wv_t = big.tile([P, N_KC, HF], BF16)
        wk_t = big.tile([P, N_KC, HF], BF16)
        wq_t = big.tile([P, N_KC, HF], BF16)
        for kc in range(N_KC):
            nc.scalar.dma_start(out=wv_t[:, kc, :], in_=wv[kc * P : (kc + 1) * P, :])
            nc.scalar.dma_start(out=wk_t[:, kc, :], in_=wk[kc * P : (kc + 1) * P, :])
            nc.scalar.dma_start(out=wq_t[:, kc, :], in_=wq[kc * P : (kc + 1) * P, :])
        wo_t = big.tile([P, N_HP, C], BF16)
        for hp in range(N_HP):
            nc.gpsimd.dma_start(out=wo_t[:, hp, :], in_=wo[hp * P : (hp + 1) * P, :])
        bq_t = big.tile([P, N_HP], F32)
        for hp in range(N_HP):
            nc.gpsimd.dma_start(
                out=bq_t[:, hp : hp + 1], in_=bq[hp * P : (hp + 1) * P].unsqueeze(-1)
            )
        bv_t = big.tile([P, HF], F32)
        nc.gpsimd.dma_start(out=bv_t[:, :], in_=bv_b[:, :])

        v_res = big.tile([P, N_TT, HH * VW], BF16)  # [v_h | 1] per head per chunk
        kt_res = big.tile([P, N_HP, T], BF16)       # K^T (f, t)
        qt = big.tile([P, N_HP, T], BF16)           # Q^T (f, q)
        attout = big.tile([P, N_HP, T], BF16)       # normalized O^T

        v_ones = v_res.rearrange("p t (h w) -> p t h w", w=VW)
        nc.vector.memset(v_ones[:, :, :, D : D + 1], 1.0)

        ones_bf = big.tile([1, 64], BF16)
        nc.vector.memset(ones_bf[:, :], 1.0)

        # ---- V = X @ Wv + bv (tokens on partitions) -------------------------
        bv_v = bv_t.rearrange("p (h d) -> p h d", h=HH)
        for tt in range(N_TT):
            pv = pr_ps.tile([P, HF], F32, tag="pr")
            for kc in range(N_KC):
                nc.tensor.matmul(
                    pv[:, :],
                    xt[:, kc, tt * P : (tt + 1) * P],
                    wv_t[:, kc, :],
                    start=(kc == 0),
                    stop=(kc == N_KC - 1),
                )
            pv_v = pv.rearrange("p (h d) -> p h d", h=HH)
            nc.vector.tensor_add(v_ones[:, tt, :, 0:D], pv_v[:, :, :], bv_v[:, :, :])

        # ---- K^T = (X @ Wk)^T per head pair (bk cancels in softmax) ---------
        for hp in range(N_HP):
            for th in range(N_QP):
                pk = pr_ps.tile([P, QW], F32, tag="pr")
                for kc in range(N_KC):
                    nc.tensor.matmul(
                        pk[:, :],
                        wk_t[:, kc, hp * P : (hp + 1) * P],
                        xt[:, kc, th * QW : (th + 1) * QW],
                        start=(kc == 0),
                        stop=(kc == N_KC - 1),
                    )
                nc.vector.tensor_copy(kt_res[:, hp, th * QW : (th + 1) * QW], pk[:, :])

        # ---- Q^T = (X @ Wq + bq)^T per head pair ----------------------------
        for hp in range(N_HP):
            for th in range(N_QP):
                pq = pr_ps.tile([P, QW], F32, tag="pr")
                for kc in range(N_KC):
                    nc.tensor.matmul(
                        pq[:, :],
                        wq_t[:, kc, hp * P : (hp + 1) * P],
                        xt[:, kc, th * QW : (th + 1) * QW],
                        start=(kc == 0),
                        stop=(kc == N_KC - 1),
                    )
                nc.vector.tensor_scalar_add(
                    qt[:, hp, th * QW : (th + 1) * QW], pq[:, :], bq_t[:, hp : hp + 1]
                )

        # ---- attention: per head pair, per query pass -----------------------
        for hp in range(N_HP):
            hA, hB = 2 * hp, 2 * hp + 1
            for qp in range(N_QP):
                q0 = qp * QW
                oA = o_ps.tile([VW, QW], F32, tag="o")
                oB = o_ps.tile([VW, QW], F32, tag="o")
                for kt in range(N_TT):
                    s = s_ps.tile([P, 2 * QW], F32, tag="s")
                    nc.tensor.matmul(
                        s[:, 0:QW],
                        kt_res[0:64, hp, kt * P : (kt + 1) * P],
                        qt[0:64, hp, q0 : q0 + QW],
                        start=True,
                        stop=True,
                        tile_position=(0, 0),
                    )
                    nc.tensor.matmul(
                        s[:, QW : 2 * QW],
                        kt_res[64:128, hp, kt * P : (kt + 1) * P],
                        qt[64:128, hp, q0 : q0 + QW],
                        start=True,
                        stop=True,
                        tile_position=(64, 0),
                    )
                    p_t = pt_pool.tile([P, 2 * QW], BF16, tag="pt")
                    nc.scalar.activation(p_t[:, :], s[:, :], AF.Exp, scale=0.125)
                    if debug and hp == 0 and qp == 0 and kt == 0:
                        dcp = bc_pool.tile([P, 2 * QW], F32, tag="dbgs")
                        nc.vector.tensor_copy(dcp[:, :], s[:, :])
                        nc.sync.dma_start(out=dbg["dbg_s"][:, :], in_=dcp[:, :])
                        nc.sync.dma_start(out=dbg["dbg_pt"][:, :], in_=p_t[:, :])
                    nc.tensor.matmul(
                        oA[:, :],
                        v_res[:, kt, hA * VW : (hA + 1) * VW],
                        p_t[:, 0:QW],
                        start=(kt == 0),
                        stop=(kt == N_TT - 1),
                    )
                    nc.tensor.matmul(
                        oB[:, :],
                        v_res[:, kt, hB * VW : (hB + 1) * VW],
                        p_t[:, QW : 2 * QW],
                        start=(kt == 0),
                        stop=(kt == N_TT - 1),
                    )
                if debug and hp == 0 and qp == 0:
                    ocp = bc_pool.tile([VW, QW], F32, tag="dbgo")
                    nc.vector.tensor_copy(ocp[:, :], oA[:, :])
                    nc.sync.dma_start(out=dbg["dbg_o"][:, :], in_=ocp[:, :])
                # normalize: attout[d, q] = O[d, q] / O[64, q]
                for row0, o_t in ((0, oA), (64, oB)):
                    den_t = rc_pool.tile([1, QW], F32, tag="den")
                    nc.vector.tensor_copy(den_t[:, :], o_t[64 : VW, :])
                    rc_t = rc_pool.tile([1, QW], F32, tag="rc")
                    nc.vector.reciprocal_approx_fast(
                        out=rc_t[:, :], in_=den_t[:, :]
                    )
                    rc_bf = rc_pool.tile([1, QW], BF16, tag="rcb")
                    nc.vector.tensor_copy(rc_bf[:, :], rc_t[:, :])
                    # broadcast the reciprocal row to 64 partitions on the PE
                    bc_ps = s_ps.tile([P, 2 * QW], F32, tag="s")
                    nc.tensor.matmul(
                        bc_ps[0:64, 0:QW],
                        ones_bf[:, :],
                        rc_bf[:, :],
                        start=True,
                        stop=True,
                    )
                    bc_t = bc_pool.tile([64, QW], F32, tag="bc")
                    nc.vector.tensor_copy(bc_t[:, :], bc_ps[0:64, 0:QW])
                    nc.vector.tensor_mul(
                        attout[row0 : row0 + 64, hp, q0 : q0 + QW],
                        o_t[0:64, :],
                        bc_t[:, :],
                    )
                    if debug and hp == 0 and qp == 0 and row0 == 0:
                        nc.sync.dma_start(out=dbg["dbg_rc"][:, :], in_=rc_t[:, :])
                        nc.sync.dma_start(out=dbg["dbg_bc"][:, :], in_=bc_t[:, :])

        if debug:
            nc.sync.dma_start(out=dbg["dbg_xt"][:, :], in_=xt[:, 0, :])
            nc.sync.dma_start(out=dbg["dbg_qt"][:, :], in_=qt[:, 0, :])
            nc.sync.dma_start(out=dbg["dbg_kt"][:, :], in_=kt_res[:, 0, :])
            nc.sync.dma_start(out=dbg["dbg_vd"][:, :], in_=v_res[:, 0, :])
            nc.sync.dma_start(out=dbg["dbg_ao"][:, :], in_=attout[:, 0, :])

        # ---- output projection: partial product over this core's features --
        for qm in range(N_TT):
            for nh in range(2):
                po = pr_ps.tile([P, QW], F32, tag="pr")
                for hp in range(N_HP):
                    nc.tensor.matmul(
                        po[:, :],
                        attout[:, hp, qm * P : (qm + 1) * P],
                        wo_t[:, hp, nh * QW : (nh + 1) * QW],
                        start=(hp == 0),
                        stop=(hp == N_HP - 1),
                    )
                os_ = bc_pool.tile([P, QW], F32, tag="os")
                nc.vector.tensor_copy(os_[:, :], po[:, :])
                nc.sync.dma_start(
                    out=out[qm, :, nh * QW : (nh + 1) * QW], in_=os_[:, :]
                )

    nc.finalize()
    return nc


def _get_program():
    if "nc" not in _CACHE:
        _CACHE["nc"] = _build()
    return _CACHE["nc"]


def _bf16(a):
    import ml_dtypes

    return np.asarray(a, np.float32).astype(ml_dtypes.bfloat16)


def kernel(x, Wq, bq, Wk, bk, Wv, bv, Wo, bo, _trace=False, _trace_kwargs=None):
    x = np.asarray(x, np.float32)
    bq, bv, bo = (np.asarray(b, np.float32) for b in (bq, bv, bo))
    # bk unused: a key-side bias adds a per-query constant to every logit of a
    # softmax row, which cancels exactly in the softmax.

    x_b = [_bf16(x[b]) for b in range(4)]
    wq_h = [_bf16(Wq[:, h * HF : (h + 1) * HF]) for h in range(2)]
    wk_h = [_bf16(Wk[:, h * HF : (h + 1) * HF]) for h in range(2)]
    wv_h = [_bf16(Wv[:, h * HF : (h + 1) * HF]) for h in range(2)]
    wo_h = [np.ascontiguousarray(_bf16(Wo[h * HF : (h + 1) * HF, :])) for h in range(2)]
    bq_h = [np.ascontiguousarray(bq[h * HF : (h + 1) * HF]) for h in range(2)]
    bv_h = [
        np.ascontiguousarray(np.broadcast_to(bv[h * HF : (h + 1) * HF], (P, HF)))
        for h in range(2)
    ]

    nc = _get_program()
    in_maps = []
    for c in range(8):
        b, hh = divmod(c, 2)
        in_maps.append(
            {
                "x": x_b[b],
                "wq": wq_h[hh], "wk": wk_h[hh], "wv": wv_h[hh],
                "wo": wo_h[hh], "bq": bq_h[hh], "bv_b": bv_h[hh],
            }
        )

    kw = {}
    if _trace:
        kw = dict(trace=True, **(_trace_kwargs or {}))
    res = run_bass_kernel_spmd(nc, in_maps, list(range(8)), **kw)
    _CACHE["last_result"] = res

    outp = np.empty((4, T, C), np.float32)
    for b in range(4):
        p0 = res.results[2 * b]["out"].reshape(T, C)
        p1 = res.results[2 * b + 1]["out"].reshape(T, C)
        outp[b] = p0 + p1
    outp += bo.astype(np.float32)
    return outp


# revision 24
# speedup vs baseline: 1.4070x; 1.0970x over previous
"""Multi-head self-attention (B=4, T=2048, C=1024, H=16) on 8 Trainium2 cores.

Sharding (head-split): core c handles batch b = c//2 and head-half
hh = c%2 (8 of the 16 heads), ALL 2048 queries and keys of its batch.
No K/V projection redundancy. The output projection contracts only this
core's 512 feature columns, so each core returns a PARTIAL [2048, 1024]
fp32 product; the host sums the two partials per batch and adds bo.

Engine plan (measured: bf16 N=512 matmul back-to-back at 216 ns with
LDWEIGHTS hidden; K=64 matmul pairs at tile_position (0,0)/(64,0) run
CONCURRENTLY; ScalarE ACTIVATE = (N+352)/1.2 ns, dtype-independent):
  - ScalarE exp() of the 33.5M logits is the pacer: 256 x [128,1024]
    ACTIVATEs ~ 294 us.
  - PE: V projection upfront; K^T/Q^T of head pair hp+1 and the output
    projection of hp-1 are INTERLEAVED into hp's attention inner loop so
    the PE never idles long enough for the HAM activity monitor to
    re-throttle the clock, and no separate projection phases remain.
  - DVE: bias adds, PSUM->SBUF casts, softmax normalize.

Layouts are feature-on-partition throughout: X^T via DMA transpose (sync
queue ONLY - transpose on the Activation queue loses the completion
ordering and races); K^T/Q^T per head pair (2x64 features on partitions
0:63/64:127); V as [key-chunk, head, 64+ones] so softmax denominators
ride along row 64 of the AV accumulation.
"""
import sys

sys.path.insert(0, "/opt/trn_rl_repo")

from contextlib import ExitStack

import numpy as np

import concourse.bacc as bacc
import concourse.tile as tile
from concourse import mybir
from concourse.bass_utils import run_bass_kernel_spmd

F32 = mybir.dt.float32
BF16 = mybir.dt.bfloat16
AF = mybir.ActivationFunctionType

T, C, NH, D = 2048, 1024, 16, 64
HH = 8                  # heads per core
HF = HH * D             # 512 feature columns per core
P = 128
N_KC = C // P           # 8 contraction chunks
N_TT = T // P           # 16 token/key chunks
N_HP = HH // 2          # 4 head pairs per core
N_QP = 4                # query passes of 512
QW = T // N_QP          # 512 queries per pass
VW = D + 1              # per-head V width incl. ones column

_CACHE = {}


def _build(debug=False):
    nc = bacc.Bacc("TRN2", target_bir_lowering=False, debug=False)

    x = nc.declare_dram_parameter("x", [T, C], BF16, isOutput=False)
    wq = nc.declare_dram_parameter("wq", [C, HF], BF16, isOutput=False)
    wk = nc.declare_dram_parameter("wk", [C, HF], BF16, isOutput=False)
    wv = nc.declare_dram_parameter("wv", [C, HF], BF16, isOutput=False)
    wo = nc.declare_dram_parameter("wo", [HF, C], BF16, isOutput=False)
    bq = nc.declare_dram_parameter("bq", [HF], F32, isOutput=False)
    bv_b = nc.declare_dram_parameter("bv_b", [P, HF], F32, isOutput=False)
    out = nc.declare_dram_parameter("out", [N_TT, P, C], F32, isOutput=True)

    dbg = {}
    if debug:
        for name, shape, dt_ in [
            ("dbg_xt", [P, T], BF16),
            ("dbg_qt", [P, T], BF16),
            ("dbg_kt", [P, T], BF16),
            ("dbg_vd", [P, HH * VW], BF16),
            ("dbg_s", [P, 2 * QW], F32),
            ("dbg_pt", [P, 2 * QW], BF16),
            ("dbg_o", [VW, QW], F32),
            ("dbg_rc", [1, QW], F32),
            ("dbg_bc", [64, QW], F32),
            ("dbg_ao", [P, T], BF16),
        ]:
            dbg[name] = nc.declare_dram_parameter(name, shape, dt_, isOutput=True)

    with tile.TileContext(nc) as tc, ExitStack() as ctx:
        big = ctx.enter_context(tc.tile_pool(name="big", bufs=1))
        pt_pool = ctx.enter_context(tc.tile_pool(name="pt", bufs=3))
        rc_pool = ctx.enter_context(tc.tile_pool(name="rc", bufs=2))
        bc_pool = ctx.enter_context(tc.tile_pool(name="bc", bufs=2))
        s_ps = ctx.enter_context(tc.tile_pool(name="sps", bufs=2, space="PSUM"))
        o_ps = ctx.enter_context(tc.tile_pool(name="ops", bufs=2, space="PSUM"))
        pr_ps = ctx.enter_context(tc.tile_pool(name="prps", bufs=2, space="PSUM"))

        # ---- inputs to SBUF -------------------------------------------------
        bq_t = big.tile([P, N_HP], F32)
        for hp in range(N_HP):
            nc.gpsimd.dma_start(
                out=bq_t[:, hp : hp + 1], in_=bq[hp * P : (hp + 1) * P].unsqueeze(-1)
            )
        bv_t = big.tile([P, HF], F32)
        nc.gpsimd.dma_start(out=bv_t[:, :], in_=bv_b[:, :])

        xt = big.tile([P, N_KC, T], BF16)          # X^T (c, t)
        for kc in range(N_KC):
            nc.sync.dma_start(
                out=xt[:, kc, :], in_=x[:, kc * P : (kc + 1) * P], transpose=True
            )

        wv_t = big.tile([P, N_KC, HF], BF16)
        wk_t = big.tile([P, N_KC, HF], BF16)
        wq_t = big.tile([P, N_KC, HF], BF16)
        wdma = [nc.scalar, nc.gpsimd]
        for kc in range(N_KC):
            wdma[kc % 2].dma_start(out=wv_t[:, kc, :], in_=wv[kc * P : (kc + 1) * P, :])
        for kc in range(N_KC):
            wdma[kc % 2].dma_start(out=wk_t[:, kc, :], in_=wk[kc * P : (kc + 1) * P, :])
        for kc in range(N_KC):
            wdma[kc % 2].dma_start(out=wq_t[:, kc, :], in_=wq[kc * P : (kc + 1) * P, :])
        wo_t = big.tile([P, N_HP, C], BF16)
        for hp in range(N_HP):
            wdma[hp % 2].dma_start(out=wo_t[:, hp, :], in_=wo[hp * P : (hp + 1) * P, :])

        v_res = big.tile([P, N_TT, HH * VW], BF16)  # [v_h | 1] per head per chunk
        kt_res = big.tile([P, N_HP, T], BF16)       # K^T (f, t)
        qt = big.tile([P, N_HP, T], BF16)           # Q^T (f, q)
        attout = big.tile([P, N_HP, T], BF16)       # normalized O^T

        v_ones = v_res.rearrange("p t (h w) -> p t h w", w=VW)
        nc.vector.memset(v_ones[:, :, :, D : D + 1], 1.0)

        ones_bf = big.tile([1, 64], BF16)
        nc.vector.memset(ones_bf[:, :], 1.0)

        # ---- V = X @ Wv + bv, all heads (tokens on partitions) --------------
        bv_v = bv_t.rearrange("p (h d) -> p h d", h=HH)
        for tt in range(N_TT):
            pv = pr_ps.tile([P, HF], F32, tag="pr")
            for kc in range(N_KC):
                nc.tensor.matmul(
                    pv[:, :],
                    xt[:, kc, tt * P : (tt + 1) * P],
                    wv_t[:, kc, :],
                    start=(kc == 0),
                    stop=(kc == N_KC - 1),
                )
            pv_v = pv.rearrange("p (h d) -> p h d", h=HH)
            nc.vector.tensor_add(v_ones[:, tt, :, 0:D], pv_v[:, :, :], bv_v[:, :, :])

        # ---- projection work generators (emitted inline with attention) ----
        def k_proj_steps(hp):
            """K^T(hp): 4 th-groups x (8 accumulating MMs + a DVE cast)."""
            for th in range(N_QP):
                pk = pr_ps.tile([P, QW], F32, tag="pr")
                for kc in range(N_KC):
                    yield lambda hp=hp, th=th, kc=kc, pk=pk: nc.tensor.matmul(
                        pk[:, :],
                        wk_t[:, kc, hp * P : (hp + 1) * P],
                        xt[:, kc, th * QW : (th + 1) * QW],
                        start=(kc == 0),
                        stop=(kc == N_KC - 1),
                    )
                yield lambda hp=hp, th=th, pk=pk: nc.vector.tensor_copy(
                    kt_res[:, hp, th * QW : (th + 1) * QW], pk[:, :]
                )

        def q_proj_steps(hp):
            for th in range(N_QP):
                pq = pr_ps.tile([P, QW], F32, tag="pr")
                for kc in range(N_KC):
                    yield lambda hp=hp, th=th, kc=kc, pq=pq: nc.tensor.matmul(
                        pq[:, :],
                        wq_t[:, kc, hp * P : (hp + 1) * P],
                        xt[:, kc, th * QW : (th + 1) * QW],
                        start=(kc == 0),
                        stop=(kc == N_KC - 1),
                    )
                yield lambda hp=hp, th=th, pq=pq: nc.vector.tensor_scalar_add(
                    qt[:, hp, th * QW : (th + 1) * QW], pq[:, :], bq_t[:, hp : hp + 1]
                )

        def out_proj_steps(qms):
            """Output projection for query chunks qms (contract all 4 hp)."""
            for qm in qms:
                for nh in range(2):
                    po = pr_ps.tile([P, QW], F32, tag="pr")
                    for hp in range(N_HP):
                        yield lambda qm=qm, nh=nh, hp=hp, po=po: nc.tensor.matmul(
                            po[:, :],
                            attout[:, hp, qm * P : (qm + 1) * P],
                            wo_t[:, hp, nh * QW : (nh + 1) * QW],
                            start=(hp == 0),
                            stop=(hp == N_HP - 1),
                        )

                    def _drain(qm=qm, nh=nh, po=po):
                        os_ = bc_pool.tile([P, QW], F32, tag="os")
                        nc.vector.tensor_copy(os_[:, :], po[:, :])
                        nc.sync.dma_start(
                            out=out[qm, :, nh * QW : (nh + 1) * QW], in_=os_[:, :]
                        )

                    yield _drain

        def chain(*gens):
            for g in gens:
                yield from g

        # upfront: K^T(0), Q^T(0) (V is already queued above)
        for step in chain(k_proj_steps(0), q_proj_steps(0)):
            step()

        # side work emitted during attention inner loops:
        #   hp 0..2: K^T/Q^T of head pair hp+1 (out-proj needs ALL head
        #   pairs, so it can only start once hp 3's attout rows land).
        #   hp 3, pass qp: out-proj of query chunks covered by pass qp-1.
        #   Tail: out-proj of the last pass's chunks (qm 12..15).
        side = {}
        for hp in range(3):
            g = chain(k_proj_steps(hp + 1), q_proj_steps(hp + 1))
            for qp in range(N_QP):
                side[(hp, qp)] = (g, 72 / 64)
        side[(3, 0)] = (iter(()), 0.0)
        for qp in range(1, N_QP):
            side[(3, qp)] = (out_proj_steps(range(4 * (qp - 1), 4 * qp)), 36 / 16)

        # ---- attention: per head pair, per query pass -----------------------
        for hp in range(N_HP):
            hA, hB = 2 * hp, 2 * hp + 1
            for qp in range(N_QP):
                gen, side_per_iter = side[(hp, qp)]
                quota = 0.0
                q0 = qp * QW
                oA = o_ps.tile([VW, QW], F32, tag="o")
                oB = o_ps.tile([VW, QW], F32, tag="o")
                for kt in range(N_TT):
                    s = s_ps.tile([P, 2 * QW], F32, tag="s")
                    nc.tensor.matmul(
                        s[:, 0:QW],
                        kt_res[0:64, hp, kt * P : (kt + 1) * P],
                        qt[0:64, hp, q0 : q0 + QW],
                        start=True,
                        stop=True,
                        tile_position=(0, 0),
                    )
                    nc.tensor.matmul(
                        s[:, QW : 2 * QW],
                        kt_res[64:128, hp, kt * P : (kt + 1) * P],
                        qt[64:128, hp, q0 : q0 + QW],
                        start=True,
                        stop=True,
                        tile_position=(64, 0),
                    )
                    p_t = pt_pool.tile([P, 2 * QW], BF16, tag="pt")
                    nc.scalar.activation(p_t[:, :], s[:, :], AF.Exp, scale=0.125)
                    if debug and hp == 0 and qp == 0 and kt == 0:
                        dcp = bc_pool.tile([P, 2 * QW], F32, tag="dbgs")
                        nc.vector.tensor_copy(dcp[:, :], s[:, :])
                        nc.sync.dma_start(out=dbg["dbg_s"][:, :], in_=dcp[:, :])
                        nc.sync.dma_start(out=dbg["dbg_pt"][:, :], in_=p_t[:, :])
                    nc.tensor.matmul(
                        oA[:, :],
                        v_res[:, kt, hA * VW : (hA + 1) * VW],
                        p_t[:, 0:QW],
                        start=(kt == 0),
                        stop=(kt == N_TT - 1),
                    )
                    nc.tensor.matmul(
                        oB[:, :],
                        v_res[:, kt, hB * VW : (hB + 1) * VW],
                        p_t[:, QW : 2 * QW],
                        start=(kt == 0),
                        stop=(kt == N_TT - 1),
                    )
                    # emit interleaved projection work
                    quota += side_per_iter
                    while quota >= 1.0:
                        step = next(gen, None)
                        if step is None:
                            quota = 0.0
                            break
                        step()
                        quota -= 1.0

                if debug and hp == 0 and qp == 0:
                    ocp = bc_pool.tile([VW, QW], F32, tag="dbgo")
                    nc.vector.tensor_copy(ocp[:, :], oA[:, :])
                    nc.sync.dma_start(out=dbg["dbg_o"][:, :], in_=ocp[:, :])
                # normalize: attout[d, q] = O[d, q] / O[64, q]
                for row0, o_t in ((0, oA), (64, oB)):
                    den_t = rc_pool.tile([1, QW], F32, tag="den")
                    nc.vector.tensor_copy(den_t[:, :], o_t[64:VW, :])
                    rc_t = rc_pool.tile([1, QW], F32, tag="rc")
                    nc.vector.reciprocal_approx_fast(out=rc_t[:, :], in_=den_t[:, :])
                    rc_bf = rc_pool.tile([1, QW], BF16, tag="rcb")
                    nc.vector.tensor_copy(rc_bf[:, :], rc_t[:, :])
                    bc_ps = s_ps.tile([P, 2 * QW], F32, tag="s")
                    nc.tensor.matmul(
                        bc_ps[0:64, 0:QW],
                        ones_bf[:, :],
                        rc_bf[:, :],
                        start=True,
                        stop=True,
                    )
                    bc_t = bc_pool.tile([64, QW], F32, tag="bc")
                    nc.vector.tensor_copy(bc_t[:, :], bc_ps[0:64, 0:QW])
                    nc.vector.tensor_mul(
                        attout[row0 : row0 + 64, hp, q0 : q0 + QW],
                        o_t[0:64, :],
                        bc_t[:, :],
                    )
                    if debug and hp == 0 and qp == 0 and row0 == 0:
                        nc.sync.dma_start(out=dbg["dbg_rc"][:, :], in_=rc_t[:, :])
                        nc.sync.dma_start(out=dbg["dbg_bc"][:, :], in_=bc_t[:, :])
                # drain leftover side work (shared gens span all 4 passes)
                if qp == N_QP - 1 or hp == 3:
                    for step in gen:
                        step()

        if debug:
            nc.sync.dma_start(out=dbg["dbg_xt"][:, :], in_=xt[:, 0, :])
            nc.sync.dma_start(out=dbg["dbg_qt"][:, :], in_=qt[:, 0, :])
            nc.sync.dma_start(out=dbg["dbg_kt"][:, :], in_=kt_res[:, 0, :])
            nc.sync.dma_start(out=dbg["dbg_vd"][:, :], in_=v_res[:, 0, :])
            nc.sync.dma_start(out=dbg["dbg_ao"][:, :], in_=attout[:, 0, :])

        # ---- output projection tail (qm 12..15; rest ran inside hp 3) -------
        for step in out_proj_steps(range(12, N_TT)):
            step()

    nc.finalize()
    return nc


def _get_program():
    if "nc" not in _CACHE:
        _CACHE["nc"] = _build()
    return _CACHE["nc"]


def _bf16(a):
    import ml_dtypes

    return np.asarray(a, np.float32).astype(ml_dtypes.bfloat16)


def kernel(x, Wq, bq, Wk, bk, Wv, bv, Wo, bo, _trace=False, _trace_kwargs=None):
    x = np.asarray(x, np.float32)
    bq, bv, bo = (np.asarray(b, np.float32) for b in (bq, bv, bo))
    # bk unused: a key-side bias adds a per-query constant to every logit of a
    # softmax row, which cancels exactly in the softmax.

    x_b = [_bf16(x[b]) for b in range(4)]
    wq_h = [_bf16(Wq[:, h * HF : (h + 1) * HF]) for h in range(2)]
    wk_h = [_bf16(Wk[:, h * HF : (h + 1) * HF]) for h in range(2)]
    wv_h = [_bf16(Wv[:, h * HF : (h + 1) * HF]) for h in range(2)]
    wo_h = [np.ascontiguousarray(_bf16(Wo[h * HF : (h + 1) * HF, :])) for h in range(2)]
    bq_h = [np.ascontiguousarray(bq[h * HF : (h + 1) * HF]) for h in range(2)]
    bv_h = [
        np.ascontiguousarray(np.broadcast_to(bv[h * HF : (h + 1) * HF], (P, HF)))
        for h in range(2)
    ]

    nc = _get_program()
    in_maps = []
    for c in range(8):
        b, hh = divmod(c, 2)
        in_maps.append(
            {
                "x": x_b[b],
                "wq": wq_h[hh], "wk": wk_h[hh], "wv": wv_h[hh],
                "wo": wo_h[hh], "bq": bq_h[hh], "bv_b": bv_h[hh],
            }
        )

    kw = {}
    if _trace:
        kw = dict(trace=True, **(_trace_kwargs or {}))
    res = run_bass_kernel_spmd(nc, in_maps, list(range(8)), **kw)
    _CACHE["last_result"] = res

    outp = np.empty((4, T, C), np.float32)
    for b in range(4):
        p0 = res.results[2 * b]["out"].reshape(T, C)
        p1 = res.results[2 * b + 1]["out"].reshape(T, C)
        outp[b] = p0 + p1
    outp += bo.astype(np.float32)
    return outp
